# revision 35
# baseline (speedup 1.0000x reference)
"""BoxFilter (9x9 box sum with edge clamping) on 8 Trainium2 NeuronCores.

Reference semantics (B, C, H, W fp32, r=4):
    out = diff_y(cumsum_W(diff_x(cumsum_H(x))))
i.e. a separable 9-wide box *sum* along H then W, with windows truncated at
the image borders.

Strategy (v4 — fp16 I/O, PE/DVE load-balanced W-box):
  - Shard data-parallel over batch: B=8 -> one (3, 1080, 1920) image per core.
  - HBM I/O in fp16 (host converts): halves DMA traffic vs fp32. The
    correctness budget (rel 2e-2 of output scale ~48) dwarfs fp16 rounding.
  - Per core, 27 tiles (3 channels x 9 blocks of 120 output rows); each
    tile loads 128 input rows (+-4 halo) x full W. The H-box is a constant
    0/1 banded matrix (top / interior / bottom variants) applied on PE.
  - W-box, two variants balanced across tiles:
      * comb path (most tiles): ones(9) = ones(3) conv {d(-3),d(0),d(+3)}.
        DVE computes box3 with two shifted tensor_tensor adds (fp16 2x_1p
        = 0.5 cyc/elem); PE applies the H-band 3x per 512-col group with
        rhs shifted by {0,3,6}, accumulating in PSUM (fp16 matmul is
        1 cyc/col, so 3 passes ~2.6us/tile).
      * scan path (a few tiles): the old DVE tensor_tensor_scan
        S[w] = S[w-1] + x[w+4] - x[w-5] (4.1us/tile, no fast mode) and a
        single PE pass. DVE cost ~2x comb, PE cost ~1/3: assigning ~3
        tiles to this path equalizes PE and DVE at ~64us each.
  - Zero-padded rows (9 left for scan warm-up, 4 right) give border
    truncation for free in both variants.
  - PSUM -> SBUF (fp32->fp16) as ONE wide ACTIVATE per tile on ScalarE
    ([120, 1920] over a 4-bank PSUM tile, bufs=2).
  - Loads: SP HWDGE ring mostly, every 3rd tile on the ACT ring; stores
    on GpSimd SWDGE; GpSimd is barred from PSUM so it only stores+memsets.
"""

import sys

if "/opt/trn_rl_repo" not in sys.path:
    sys.path.insert(0, "/opt/trn_rl_repo")

import numpy as np

B, C, H, W = 8, 3, 1080, 1920
R = 4
BLK = 120          # output rows per tile
NBLK = H // BLK    # 9
LP = 9             # left zero pad (scan warm-up needs 2r+1)
RP = 4             # right zero pad
XW = LP + W + RP   # padded row width (1933)
W3 = W + 6         # box3 row width: box3[w] for w in [-3, W+2]
SCN = W + R        # scan length; outputs [R:] are S[0..W-1]
N_TILES = C * NBLK
# tiles that take the scan path (DVE-heavy, PE-light) to balance engines.
# NOTE: offloading adds to GpSimd regressed badly — its DSP activity trips
# the chip power throttle (half-clock 30% of the time); keep GpSimd to
# stores/memsets only.
SCAN_TILES = frozenset({4, 13, 22})
# ramp/tail tiles whose evac+store run as two W-halves to shorten the
# serial chain at the pipeline's start and end
SPLIT_TILES = frozenset({0, 1, N_TILES - 1})


def _band_matrices() -> np.ndarray:
    """[128, 3*BLK] fp16: the three 0/1 banded H-box matrices, side by side.

    out[m, n] = sum_k band[k, m] * in[k, n]; column m holds the taps for
    output row m of the block.
    """
    b0 = np.zeros((128, BLK), np.float16)   # first block: rows 0..127 loaded
    b1 = np.zeros((128, BLK), np.float16)   # interior: rows h0-4..h0+123
    b2 = np.zeros((128, BLK), np.float16)   # last block: rows H-128..H-1
    for m in range(BLK):
        b0[max(0, m - R): m + R + 1, m] = 1.0
        b1[m: m + 2 * R + 1, m] = 1.0
        b2[m + R: min(m + 3 * R, 127) + 1, m] = 1.0
    return np.concatenate([b0, b1, b2], axis=1)


def _build_nc():
    import concourse.tile as tile
    from concourse import bacc, mybir

    f16 = mybir.dt.float16
    nc = bacc.Bacc("TRN2", target_bir_lowering=False, debug=False)
    x_d = nc.dram_tensor("x", [C, H, W], f16, kind="ExternalInput").ap()
    out_d = nc.dram_tensor("out", [C, H, W], f16, kind="ExternalOutput").ap()
    bands_d = nc.inline_tensor(_band_matrices(), name="bands").ap()

    with tile.TileContext(nc) as tc:
        _tile_body(tc, out_d, x_d, bands_d, mybir)
    nc.compile()
    return nc


def _tile_body(tc, out_d, x_d, bands_d, mybir):
    nc = tc.nc
    add = mybir.AluOpType.add
    sub = mybir.AluOpType.subtract
    f16 = mybir.dt.float16
    f32 = mybir.dt.float32

    with (
        tc.tile_pool(name="bands", bufs=1) as bands_pool,
        tc.tile_pool(name="xp", bufs=12) as xpool,
        tc.tile_pool(name="t1", bufs=4) as tpool,
        tc.tile_pool(name="wb", bufs=8) as wpool,
        tc.tile_pool(name="ot", bufs=8) as opool,
        # two [BLK,1024] psum tiles per iteration; bufs counts iterations
        # in flight, so 2 x 2 x 2 banks = all 8 PSUM banks
        tc.tile_pool(name="ps", bufs=2, space="PSUM") as pspool,
    ):
        bands = bands_pool.tile([128, 3 * BLK], f16)
        first = True
        tile_idx = 0

        for c in range(C):
            for t in range(NBLK):
                h0 = t * BLK
                if t == 0:
                    r0, bi = 0, 0
                elif t == NBLK - 1:
                    r0, bi = H - 128, 2
                else:
                    r0, bi = h0 - R, 1

                xp = xpool.tile([128, XW], f16)
                if tile_idx < 12:
                    # pool buffers rotate round-robin; pads stay zero after
                    # the first pass since DMA only writes the middle.
                    # DVE, not GpSimd: GpSimd's preamble table-load ends
                    # late and would gate the first input loads
                    nc.vector.memset(xp[:, 0:LP], 0.0)
                    nc.vector.memset(xp[:, LP + W: XW], 0.0)
                if tile_idx < 2:
                    # ramp: split the first loads across both rings so the
                    # first tile's data lands in ~half the time
                    nc.sync.dma_start(
                        out=xp[0:64, LP: LP + W], in_=x_d[c, r0: r0 + 64, :]
                    )
                    nc.scalar.dma_start(
                        out=xp[64:128, LP: LP + W],
                        in_=x_d[c, r0 + 64: r0 + 128, :],
                    )
                else:
                    # steady state: all loads on the SP ring (it has headroom;
                    # keeping ACT evac-only avoids queue-coupled stalls there)
                    nc.sync.dma_start(
                        out=xp[:, LP: LP + W], in_=x_d[c, r0: r0 + 128, :]
                    )
                if first:
                    nc.sync.dma_start(out=bands[:, :], in_=bands_d[:, :])
                    first = False

                band = bands[:, bi * BLK: (bi + 1) * BLK]
                ot = opool.tile([BLK, W], f16)
                split = tile_idx in SPLIT_TILES
                # matmul outputs must stay PSUM-bank-aligned (512 fp32), so
                # split tiles halve at 1024, matching chunk-group boundaries.
                # Split tiles use TWO psum tiles: evac dependencies are
                # tracked per psum buffer, so a half in its own buffer can
                # evacuate after 6 matmuls instead of 12 (shorter ramp/tail).
                bounds = (0, 512, 1024, 1536, W)
                psA = pspool.tile([BLK, 1024], f32)
                psB = pspool.tile([BLK, 1024], f32)

                def pdst(n0, nw):
                    return (
                        psA[:, n0: n0 + nw]
                        if n0 < 1024
                        else psB[:, n0 - 1024: n0 - 1024 + nw]
                    )

                if tile_idx in SCAN_TILES:
                    # scan path: W-box in one DVE scan, one PE pass
                    wb = wpool.tile([128, W3], f16)
                    nc.vector.tensor_tensor_scan(
                        out=wb[:, 0:SCN],
                        data0=xp[:, LP: LP + SCN],
                        data1=xp[:, 0:SCN],
                        initial=0.0,
                        op0=add,
                        op1=sub,
                    )
                    for q in range(4):
                        n0, nw = bounds[q], bounds[q + 1] - bounds[q]
                        nc.tensor.matmul(
                            out=pdst(n0, nw),
                            lhsT=band,
                            rhs=wb[:, R + n0: R + n0 + nw],
                            start=True,
                            stop=True,
                        )
                else:
                    # comb path: box3 on DVE (2 adds), 3 shifted PE passes
                    # wb[:, j] = box3 at w=j-3  =  x[j-4] + x[j-3] + x[j-2]
                    #          = xp[:, j+5] + xp[:, j+6] + xp[:, j+7]
                    t1 = tpool.tile([128, W3], f16)
                    wb = wpool.tile([128, W3], f16)
                    # ramp tiles: adds in halves so the first matmul group
                    # (which only needs wb[0:518]) starts ~1us earlier
                    add_ranges = (
                        ((0, 972), (972, W3)) if tile_idx < 2 else ((0, W3),)
                    )
                    for jl, jh in add_ranges:
                        nc.vector.tensor_tensor(
                            out=t1[:, jl:jh], in0=xp[:, 5 + jl: 5 + jh],
                            in1=xp[:, 6 + jl: 6 + jh], op=add,
                        )
                        nc.vector.tensor_tensor(
                            out=wb[:, jl:jh], in0=t1[:, jl:jh],
                            in1=xp[:, 7 + jl: 7 + jh], op=add,
                        )
                    for q in range(4):
                        n0, nw = bounds[q], bounds[q + 1] - bounds[q]
                        for si, s in enumerate((0, 3, 6)):
                            nc.tensor.matmul(
                                out=pdst(n0, nw),
                                lhsT=band,
                                rhs=wb[:, n0 + s: n0 + s + nw],
                                start=(si == 0),
                                stop=(si == 2),
                            )

                # PSUM->SBUF (fp32->fp16) evacuation on ScalarE, then store.
                # Stores go on GpSimd SWDGE: keeps both HWDGE rings for
                # loads. Ramp/tail tiles evac+store in halves; the final
                # tile's halves use the (by then idle) HWDGE rings so the
                # kernel end isn't behind the SWDGE backlog+drain.
                last = tile_idx == N_TILES - 1
                # half-granular evac from the two psum tiles: each half
                # evacuates after its own 6 matmuls instead of all 12
                nc.scalar.copy(out=ot[:, 0:1024], in_=psA[:, :])
                nc.scalar.copy(out=ot[:, 1024:W], in_=psB[:, 0: W - 1024])
                # all stores on the HWDGE rings (alternating): the DMA
                # queues have ample headroom and dropping SWDGE removes
                # GpSimd's expensive end-of-kernel dge_drain from the tail.
                # Store triggers wait only on the evac that ScalarE just
                # finished, so they don't stall either engine's queue.
                if split:
                    for hi_, (lo, hi) in enumerate(((0, 1024), (1024, W))):
                        st_eng = nc.sync if hi_ == 0 else nc.scalar
                        st_eng.dma_start(
                            out=out_d[c, h0: h0 + BLK, lo:hi],
                            in_=ot[:, lo:hi],
                        )
                else:
                    st_eng = nc.sync if tile_idx % 2 == 0 else nc.scalar
                    st_eng.dma_start(
                        out=out_d[c, h0: h0 + BLK, :], in_=ot[:, :]
                    )
                tile_idx += 1


_NC = None


def _get_nc():
    global _NC
    if _NC is None:
        _NC = _build_nc()
    return _NC


def run(x: np.ndarray, trace: bool = False, trace_cores=None):
    """Run the kernel on all 8 cores. Returns (out, BassKernelResults)."""
    from concourse.bass_utils import run_bass_kernel_spmd

    nc = _get_nc()
    x = np.asarray(x)
    assert x.shape == (B, C, H, W), x.shape
    x16 = np.ascontiguousarray(x.astype(np.float16))
    in_maps = [{"x": x16[b]} for b in range(B)]
    if trace and trace_cores is None:
        trace_cores = [0, 7]
    res = run_bass_kernel_spmd(
        nc, in_maps, core_ids=list(range(B)), trace=trace, trace_cores=trace_cores
    )
    out = np.stack([res.results[b]["out"] for b in range(B)], axis=0)
    return out.astype(np.float32), res


def kernel(x: np.ndarray, r) -> np.ndarray:
    assert int(np.asarray(r)) == R, f"kernel hardcodes r={R}, got {r}"
    out, _ = run(x, trace=False)
    return out
